# revision 1
# baseline (speedup 1.0000x reference)
"""Decision Transformer on 8 Trainium2 NeuronCores (Bass/Tile).

Sharding: data-parallel over batch (B=32 -> 4 seqs/core), no collectives.
Layout: feature-major activations [H, tokens]; float32r matmuls (N>=256),
bf16 for attention/proj/mlp paths.
"""
import os
import numpy as np
import ml_dtypes

import concourse.bass as bass
import concourse.tile as tile
from concourse import bacc, mybir
from concourse.bass_utils import run_bass_kernel_spmd

AF = mybir.ActivationFunctionType
ALU = mybir.AluOpType
F32 = mybir.dt.float32
F32R = mybir.dt.float32r
BF16 = mybir.dt.bfloat16

# Model dims (hardcoded per contest spec)
H, L, NHEAD, V, MAXEP = 768, 12, 12, 1654, 50
B, S, TS, TA = 32, 50, 60, 8
T = 3 * S                    # 150 tokens per seq
NC_ = 8                      # cores
BC = B // NC_                # 4 seqs per core
NTOK = BC * T                # 600 transformer tokens per core
NG = BC * S                  # 200 GRU chains per core
NGP = 256                    # padded chains (f32r wants N>=256)
HC = H // 128                # 6 feature chunks
G3 = 3 * H                   # 2304
GC = G3 // 128               # 18 gate chunks
FF = 4 * H                   # 3072
FC = FF // 128               # 24
HD = H // NHEAD              # 64
VCH = [414, 414, 414, 412]   # vocab N-chunks
TOKC = [(0, 128), (128, 22)]  # per-seq token chunks of 150

bf16 = ml_dtypes.bfloat16

_CACHED = {}


def _build():
    nc = bacc.Bacc("TRN2", target_bir_lowering=False, debug=False)
    D = {}

    def din(name, shape, dt):
        D[name] = nc.dram_tensor(name, shape, dt, kind="ExternalInput").ap()
        return D[name]

    # ---- DRAM inputs ----
    din("xg_s", [TS, G3, NG], mybir.dt.bfloat16)
    din("xg_a", [TA, G3, NG], mybir.dt.bfloat16)
    din("w_hhT", [H, G3], F32R)
    din("b_hh", [G3], F32)
    din("teT", [H, NG], F32)      # time_emb gathered, transposed
    din("terT", [H, NG], F32)     # teT + ret_b (for R tokens)
    din("rtg", [1, NG], F32)
    din("ret_w", [H], F32)
    din("wpeT", [H, T], F32)
    din("lne_g", [H], F32)
    din("lne_b", [H], F32)
    din("maskT", [BC, T, T], BF16)
    din("attn_wf", [L, H, G3], F32R)
    din("attn_bf", [L, G3], F32)
    din("attn_pw16", [L, H, H], BF16)
    din("attn_pb", [L, H], F32)
    din("fc_wf", [L, H, FF], F32R)
    din("fc_bf", [L, FF], F32)
    din("mlp_pw16", [L, FF, H], BF16)
    din("mlp_pb", [L, H], F32)
    din("head_wf", [4, H, V], F32R)
    din("head_bf", [4, V], F32)
    din("ones_r", [128, NGP], F32R)
    din("zeros_r", [128, HC, NGP], F32R)
    din("ones_b", [128, 128], BF16)
    din("ident_b", [128, 128], BF16)
    outs = [nc.dram_tensor(f"o{k}", [NG, V], F32, kind="ExternalOutput").ap()
            for k in range(4)]

    with tile.TileContext(nc) as tc:
        _body(tc, D, outs)
    nc.compile()
    return nc


def _body(tc, D, outs):
    nc = tc.nc
    from contextlib import ExitStack
    est = ExitStack()
    with est:
        persist = est.enter_context(tc.tile_pool(name="persist", bufs=1))
        sqp = est.enter_context(tc.tile_pool(name="sqp", bufs=1))
        bca = est.enter_context(tc.tile_pool(name="bca", bufs=2))
        bvec = est.enter_context(tc.tile_pool(name="bvec", bufs=2))

        # ---- persistent small tiles ----
        ones_r = persist.tile([128, NGP], F32R, tag="ones_r")
        nc.sync.dma_start(ones_r[:], D["ones_r"])
        ones_b = persist.tile([128, 128], BF16, tag="ones_b")
        nc.sync.dma_start(ones_b[:], D["ones_b"])
        ident = persist.tile([128, 128], BF16, tag="ident")
        nc.sync.dma_start(ident[:], D["ident_b"])
        bhh = persist.tile([128, GC], F32, tag="bhh")
        nc.sync.dma_start(bhh[:], D["b_hh"].rearrange("(m p) -> p m", p=128))
        maskTs = persist.tile([128, BC, 2, T], BF16, tag="maskTs")
        for b in range(BC):
            for kc, (k0, ksz) in enumerate(TOKC):
                nc.sync.dma_start(maskTs[:ksz, b, kc, :],
                                  D["maskT"][b, k0:k0 + ksz, :])
        eps_t = persist.tile([128, 1], F32, tag="eps_t")
        nc.vector.memset(eps_t[:], 1e-5)
        x_resid = persist.tile([128, HC, NTOK], F32R, tag="x_resid")

        # =============== Phase A: GRUs + assembly ===============
        with tc.tile_pool(name="hp", bufs=1) as hp:
            teT = hp.tile([128, HC, NG], F32, tag="teT")
            nc.sync.dma_start(teT[:], D["teT"].rearrange("(c p) j -> p c j", p=128))
            terT = hp.tile([128, HC, NG], F32, tag="terT")
            nc.sync.dma_start(terT[:], D["terT"].rearrange("(c p) j -> p c j", p=128))
            retw = hp.tile([128, HC], F32, tag="retw")
            nc.sync.dma_start(retw[:], D["ret_w"].rearrange("(c p) -> p c", p=128))
            rtg_bc = hp.tile([128, NG], F32, tag="rtg_bc")
            nc.gpsimd.dma_start(
                out=rtg_bc[:],
                in_=bass.AP(tensor=D["rtg"].tensor, offset=D["rtg"].offset,
                            ap=[[0, 128]] + D["rtg"].ap[1:]))
            wpeT = hp.tile([128, HC, T], F32, tag="wpeT")
            nc.sync.dma_start(wpeT[:], D["wpeT"].rearrange("(c p) t -> p c t", p=128))
            lneg = hp.tile([128, HC], F32, tag="lneg")
            nc.sync.dma_start(lneg[:], D["lne_g"].rearrange("(c p) -> p c", p=128))
            lneb = hp.tile([128, HC], F32, tag="lneb")
            nc.sync.dma_start(lneb[:], D["lne_b"].rearrange("(c p) -> p c", p=128))
            hs = [hp.tile([128, HC, NGP], F32R, tag=f"hs{i}", name=f"hs{i}")
                  for i in range(2)]
            ha = [hp.tile([128, HC, NGP], F32R, tag=f"ha{i}", name=f"ha{i}")
                  for i in range(2)]
            for t_ in hs + ha:
                nc.sync.dma_start(t_[:], D["zeros_r"])

            with tc.tile_pool(name="gruw", bufs=1) as gruw, \
                 tc.tile_pool(name="xgp", bufs=2) as xgp, \
                 tc.tile_pool(name="gtmp", bufs=1) as gtmp, \
                 tc.tile_pool(name="gps", bufs=6, space="PSUM") as gps:

                whh = gruw.tile([128, HC, G3], F32R, tag="whh")
                nc.sync.dma_start(whh[:],
                                  D["w_hhT"].rearrange("(c p) g -> p c g", p=128))

                def gru_step(t, xg_dram, hpair):
                    hcur, hnxt = hpair[t % 2], hpair[(t + 1) % 2]
                    xg = xgp.tile([128, GC, NG], BF16, tag="xg", bufs=2)
                    nc.sync.dma_start(
                        xg[:], xg_dram[t].rearrange("(m p) j -> p m j", p=128))
                    rz = {}
                    for m in range(12):  # r (0..5), z (6..11)
                        ps = gps.tile([128, NGP], F32, tag="g", bufs=6)
                        nc.tensor.matmul(ps[:, :NG], ident[:], xg[:, m, :],
                                         start=True, stop=(t == 0))
                        if t > 0:
                            for k in range(HC):
                                nc.tensor.matmul(
                                    ps[:], whh[:, k, m * 128:(m + 1) * 128],
                                    hcur[:, k, :], start=False,
                                    stop=(k == HC - 1))
                        g = gtmp.tile([128, NG], F32, tag=f"rz{m}", bufs=1)
                        nc.scalar.activation(g[:], ps[:, :NG], AF.Sigmoid,
                                             bias=bhh[:, m:m + 1])
                        rz[m] = g
                    for c in range(HC):  # n chunks (m = 12+c) + h update
                        m = 12 + c
                        t1 = gtmp.tile([128, NG], F32, tag="t1", bufs=2)
                        if t > 0:
                            ps = gps.tile([128, NGP], F32, tag="g", bufs=6)
                            for k in range(HC):
                                nc.tensor.matmul(
                                    ps[:], whh[:, k, m * 128:(m + 1) * 128],
                                    hcur[:, k, :], start=(k == 0),
                                    stop=(k == HC - 1))
                            # t1 = (hn + bhh_n) * r
                            nc.vector.scalar_tensor_tensor(
                                t1[:], ps[:, :NG], bhh[:, m:m + 1], rz[c][:],
                                ALU.add, ALU.mult)
                        else:
                            # t1 = r * bhh_n  (h = 0)
                            nc.vector.tensor_scalar_mul(t1[:], rz[c][:],
                                                        bhh[:, m:m + 1])
                        t2 = gtmp.tile([128, NG], F32, tag="t2", bufs=2)
                        nc.vector.tensor_add(t2[:], t1[:], xg[:, m, :])
                        n_ = gtmp.tile([128, NG], F32, tag="n", bufs=2)
                        nc.scalar.activation(n_[:], t2[:], AF.Tanh)
                        d = gtmp.tile([128, NG], F32, tag="d", bufs=2)
                        nc.vector.tensor_sub(d[:], hcur[:, c, :NG], n_[:])
                        e = gtmp.tile([128, NG], F32, tag="e", bufs=2)
                        nc.vector.tensor_mul(e[:], rz[6 + c][:], d[:])
                        nc.vector.tensor_add(hnxt[:, c, :NG], n_[:], e[:])

                for t in range(TS):
                    gru_step(t, D["xg_s"], hs)
                    if t < TA:
                        gru_step(t, D["xg_a"], ha)
            enc_s = hs[TS % 2]
            enc_a = ha[TA % 2]

            # ---- assemble x = interleave(R, s, a); ln_e; + wpe ----
            with tc.tile_pool(name="sst", bufs=1, space="PSUM") as sps:
                xv = x_resid[:].rearrange("p c (b s three) -> p c b three s",
                                          b=BC, three=3)
                for c in range(HC):
                    hsv = enc_s[:, c, :NG].rearrange("p (b s) -> p b s", b=BC)
                    hav = enc_a[:, c, :NG].rearrange("p (b s) -> p b s", b=BC)
                    tev = teT[:, c, :].rearrange("p (b s) -> p b s", b=BC)
                    trv = terT[:, c, :].rearrange("p (b s) -> p b s", b=BC)
                    rgv = rtg_bc[:].rearrange("p (b s) -> p b s", b=BC)
                    nc.vector.scalar_tensor_tensor(
                        xv[:, c, :, 0, :], rgv, retw[:, c:c + 1], trv,
                        ALU.mult, ALU.add)
                    nc.vector.tensor_add(xv[:, c, :, 1, :], hsv, tev)
                    nc.vector.tensor_add(xv[:, c, :, 2, :], hav, tev)
                mean, rs = _ln_stats(tc, nc, sps, bca, sqp, ones_r, x_resid,
                                     NTOK, eps_t)
                for c in range(HC):
                    tt = sqp.tile([128, NTOK], F32, tag="ln_t", bufs=2)
                    nc.vector.tensor_sub(tt[:], x_resid[:, c, :], mean[:])
                    nc.vector.tensor_mul(tt[:], tt[:], rs[:])
                    nc.scalar.activation(x_resid[:, c, :], tt[:], AF.Identity,
                                         bias=lneb[:, c:c + 1],
                                         scale=lneg[:, c:c + 1])
                    xb = x_resid[:, c, :].rearrange("p (b t) -> p b t", b=BC)
                    for b in range(BC):
                        nc.vector.tensor_add(xb[:, b, :], xb[:, b, :],
                                             wpeT[:, c, :])

        # =============== Phase B: transformer layers ===============
        with tc.tile_pool(name="actB", bufs=1) as actB, \
             tc.tile_pool(name="wcp", bufs=1) as wcp, \
             tc.tile_pool(name="att_sb", bufs=1) as att_sb:
            for l in range(L):
                ab = bvec.tile([128, 12], F32, tag="ab")
                nc.sync.dma_start(ab[:], D["attn_bf"][l, :12 * 128]
                                  .rearrange("(m p) -> p m", p=128))
                avb = bvec.tile([1, H], F32R, tag="avb")
                nc.sync.dma_start(
                    avb[:], D["attn_bf"][l, 2 * H:3 * H][None, :].bitcast(F32R))
                pb = bvec.tile([128, HC], F32, tag="pb")
                nc.sync.dma_start(pb[:],
                                  D["attn_pb"][l].rearrange("(m p) -> p m", p=128))
                fcb = bvec.tile([128, FC], F32, tag="fcb")
                nc.sync.dma_start(fcb[:],
                                  D["fc_bf"][l].rearrange("(m p) -> p m", p=128))
                mpb = bvec.tile([128, HC], F32, tag="mpb")
                nc.sync.dma_start(mpb[:],
                                  D["mlp_pb"][l].rearrange("(m p) -> p m", p=128))

                # ---- LN1 ----
                with tc.tile_pool(name="st1", bufs=1, space="PSUM") as sps:
                    mean, rs = _ln_stats(tc, nc, sps, bca, sqp, ones_r,
                                         x_resid, NTOK, eps_t)
                y = actB.tile([128, HC, NTOK], F32R, tag="y")
                for c in range(HC):
                    tt = sqp.tile([128, NTOK], F32, tag="ln_t", bufs=2)
                    nc.vector.tensor_sub(tt[:], x_resid[:, c, :], mean[:])
                    nc.vector.tensor_mul(y[:, c, :], tt[:], rs[:])

                # ---- qk (column-streamed weights) + v_tok ----
                qk = actB.tile([128, 12, NTOK], BF16, tag="qk")
                awf = D["attn_wf"][l].rearrange("(k p) g -> p k g", p=128)
                with tc.tile_pool(name="mm1", bufs=1, space="PSUM") as mmp:
                    for m in range(12):
                        wc = wcp.tile([128, HC, 128], F32R, tag="wc", bufs=4,
                                      name=f"wqk{l}_{m}")
                        nc.sync.dma_start(wc[:],
                                          awf[:, :, m * 128:(m + 1) * 128])
                        for nh in range(2):
                            nsl = slice(nh * 300, nh * 300 + 300)
                            ps = mmp.tile([128, 300], F32, tag="mm", bufs=6)
                            for k in range(HC):
                                nc.tensor.matmul(ps[:], wc[:, k, :],
                                                 y[:, k, nsl], start=(k == 0),
                                                 stop=(k == HC - 1))
                            nc.scalar.activation(qk[:, m, nsl], ps[:],
                                                 AF.Identity,
                                                 bias=ab[:, m:m + 1])
                    # v token-major [tok, H] per seq
                    wv = wcp.tile([128, HC, H], F32R, tag="wv", bufs=1,
                                  name=f"wv{l}")
                    nc.sync.dma_start(wv[:], awf[:, :, 2 * H:3 * H])
                    vtok = actB.tile([128, BC, 2, H], BF16, tag="vtok")
                    for b in range(BC):
                        for kc, (k0, ksz) in enumerate(TOKC):
                            tsl = slice(b * T + k0, b * T + k0 + ksz)
                            for nh2 in range(2):
                                nsl2 = slice(nh2 * 384, nh2 * 384 + 384)
                                ps = mmp.tile([128, 384], F32, tag="mmv",
                                              bufs=2)
                                for k in range(HC):
                                    nc.tensor.matmul(
                                        ps[:ksz, :], y[:, k, tsl],
                                        wv[:, k, nsl2],
                                        start=(k == 0), stop=False)
                                nc.tensor.matmul(ps[:ksz, :],
                                                 ones_r[0:1, :ksz],
                                                 avb[:, nsl2], start=False,
                                                 stop=True)
                                nc.scalar.copy(vtok[:ksz, b, kc, nsl2],
                                               ps[:ksz, :])

                # ---- attention (bf16): sT -> exp -> mask -> Z -> oT ----
                ox = actB.tile([128, HC, NTOK], BF16, tag="ox")
                with tc.tile_pool(name="aps", bufs=1, space="PSUM") as aps:
                    for b in range(BC):
                        for h in range(NHEAD):
                            p0 = (h % 2) * 64
                            qh = qk[p0:p0 + 64, h // 2, b * T:(b + 1) * T]
                            kh = qk[p0:p0 + 64, 6 + h // 2, b * T:(b + 1) * T]
                            attm = []
                            psz = aps.tile([128, T], F32, tag="z", bufs=2)
                            for kc, (k0, ksz) in enumerate(TOKC):
                                pss = aps.tile([128, T], F32, tag="s", bufs=4)
                                nc.tensor.matmul(pss[:ksz, :],
                                                 kh[:, k0:k0 + ksz], qh[:],
                                                 start=True, stop=True)
                                au = att_sb.tile([128, T], BF16, tag="au",
                                                 bufs=4)
                                nc.scalar.activation(au[:ksz, :], pss[:ksz, :],
                                                     AF.Exp, scale=0.125)
                                am = att_sb.tile([128, T], BF16, tag="am",
                                                 bufs=4)
                                nc.vector.tensor_mul(am[:ksz, :], au[:ksz, :],
                                                     maskTs[:ksz, b, kc, :])
                                attm.append(am)
                                nc.tensor.matmul(psz[:], ones_b[:ksz, :],
                                                 am[:ksz, :], start=(kc == 0),
                                                 stop=(kc == 1))
                            rz_ = att_sb.tile([128, T], F32, tag="rz", bufs=2)
                            nc.vector.reciprocal(rz_[:], psz[:])
                            pso = aps.tile([64, T], F32, tag="o", bufs=2)
                            for kc, (k0, ksz) in enumerate(TOKC):
                                af = att_sb.tile([128, T], BF16, tag="af",
                                                 bufs=4)
                                nc.vector.tensor_mul(af[:ksz, :],
                                                     attm[kc][:ksz, :],
                                                     rz_[:ksz, :])
                                nc.tensor.matmul(
                                    pso[:],
                                    vtok[:ksz, b, kc, h * 64:(h + 1) * 64],
                                    af[:ksz, :], start=(kc == 0),
                                    stop=(kc == 1))
                            nc.scalar.copy(
                                ox[p0:p0 + 64, h // 2, b * T:(b + 1) * T],
                                pso[:])

                # ---- proj (bf16, column-streamed) + residual ----
                apw = D["attn_pw16"][l].rearrange("(k p) g -> p k g", p=128)
                with tc.tile_pool(name="mm2", bufs=1, space="PSUM") as mmp:
                    for m in range(HC):
                        wcb = wcp.tile([128, HC, 128], BF16, tag="wcb", bufs=4,
                                       name=f"wpj{l}_{m}")
                        nc.sync.dma_start(wcb[:],
                                          apw[:, :, m * 128:(m + 1) * 128])
                        for nh in range(2):
                            nsl = slice(nh * 300, nh * 300 + 300)
                            ps = mmp.tile([128, 300], F32, tag="mm", bufs=6)
                            for k in range(HC):
                                nc.tensor.matmul(ps[:], wcb[:, k, :],
                                                 ox[:, k, nsl], start=(k == 0),
                                                 stop=(k == HC - 1))
                            nc.vector.scalar_tensor_tensor(
                                x_resid[:, m, nsl], ps[:], pb[:, m:m + 1],
                                x_resid[:, m, nsl], ALU.add, ALU.add)

                # ---- LN2 ----
                with tc.tile_pool(name="st2", bufs=1, space="PSUM") as sps:
                    mean2, rs2 = _ln_stats(tc, nc, sps, bca, sqp, ones_r,
                                           x_resid, NTOK, eps_t,
                                           psum_tag="st2")
                y2 = actB.tile([128, HC, NTOK], F32R, tag="y")
                for c in range(HC):
                    tt = sqp.tile([128, NTOK], F32, tag="ln_t", bufs=2)
                    nc.vector.tensor_sub(tt[:], x_resid[:, c, :], mean2[:])
                    nc.vector.tensor_mul(y2[:, c, :], tt[:], rs2[:])

                # ---- MLP: fc (col-streamed f32r) -> gelu -> proj (bf16) ----
                gel = actB.tile([128, FC, NTOK], BF16, tag="gel")
                fcw = D["fc_wf"][l].rearrange("(k p) g -> p k g", p=128)
                with tc.tile_pool(name="mm3", bufs=1, space="PSUM") as mmp:
                    for m in range(FC):
                        wc = wcp.tile([128, HC, 128], F32R, tag="wc", bufs=4,
                                      name=f"wfc{l}_{m}")
                        nc.sync.dma_start(wc[:],
                                          fcw[:, :, m * 128:(m + 1) * 128])
                        for nh in range(2):
                            nsl = slice(nh * 300, nh * 300 + 300)
                            ps = mmp.tile([128, 300], F32, tag="mm", bufs=6)
                            for k in range(HC):
                                nc.tensor.matmul(ps[:], wc[:, k, :],
                                                 y2[:, k, nsl], start=(k == 0),
                                                 stop=(k == HC - 1))
                            nc.scalar.activation(gel[:, m, nsl], ps[:],
                                                 AF.Gelu_apprx_tanh,
                                                 bias=fcb[:, m:m + 1])
                    mpw = D["mlp_pw16"][l].rearrange("(k p) g -> p k g", p=128)
                    for nh in range(2):
                        nsl = slice(nh * 300, nh * 300 + 300)
                        pss = [mmp.tile([128, 300], F32, tag="mm", bufs=6,
                                        name=f"mpps{l}_{nh}_{i}")
                               for i in range(HC)]
                        for k in range(FC):
                            wmt = wcp.tile([128, H], BF16, tag="wm", bufs=4,
                                           name=f"wml{l}_{nh}_{k}")
                            nc.sync.dma_start(wmt[:], mpw[:, k, :])
                            for m in range(HC):
                                nc.tensor.matmul(
                                    pss[m][:], wmt[:, m * 128:(m + 1) * 128],
                                    gel[:, k, nsl], start=(k == 0),
                                    stop=(k == FC - 1))
                        for m in range(HC):
                            nc.vector.scalar_tensor_tensor(
                                x_resid[:, m, nsl], pss[m][:],
                                mpb[:, m:m + 1], x_resid[:, m, nsl],
                                ALU.add, ALU.add)

        # =============== Phase C: lnf on state cols + heads ===============
        with tc.tile_pool(name="phC", bufs=1) as phC:
            x1v = x_resid[:].rearrange("p c (b s three) -> p c b three s",
                                       b=BC, three=3)
            x1 = phC.tile([128, HC, NG], F32R, tag="x1")
            with tc.tile_pool(name="hps", bufs=1, space="PSUM") as sps:
                psS = sps.tile([128, NG], F32, tag="hS")
                psQ = sps.tile([128, NG], F32, tag="hQ")
                for c in range(HC):
                    sq = sqp.tile([128, BC, S], F32R, tag="hsq", bufs=2)
                    nc.scalar.activation(sq[:], x1v[:, c, :, 1, :], AF.Square)
                    nc.tensor.matmul(psS[:], ones_r[:, :128],
                                     x1v[:, c, :, 1, :],
                                     start=(c == 0), stop=(c == HC - 1))
                    nc.tensor.matmul(psQ[:], ones_r[:, :128], sq[:],
                                     start=(c == 0), stop=(c == HC - 1))
                mean = bca.tile([128, NG], F32, tag="hmean")
                rs = bca.tile([128, NG], F32, tag="hrs")
                nc.scalar.mul(mean[:], psS[:], 1.0 / H)
                m2 = bca.tile([128, NG], F32, tag="hm2")
                nc.scalar.activation(m2[:], psS[:], AF.Square, scale=1.0 / H)
                vv = bca.tile([128, NG], F32, tag="hvv")
                nc.vector.scalar_tensor_tensor(vv[:], psQ[:], 1.0 / H, m2[:],
                                               ALU.mult, ALU.subtract)
                sd = bca.tile([128, NG], F32, tag="hsd")
                nc.scalar.activation(sd[:], vv[:], AF.Sqrt, bias=eps_t[:])
                nc.vector.reciprocal(rs[:], sd[:])
                for c in range(HC):
                    tt = sqp.tile([128, NG], F32, tag="hln_t", bufs=2)
                    nc.vector.tensor_sub(tt[:], x1v[:, c, :, 1, :], mean[:])
                    nc.vector.tensor_mul(x1[:, c, :], tt[:], rs[:])

            with tc.tile_pool(name="ops", bufs=1, space="PSUM") as ops:
                for hd_ in range(4):
                    hb = bvec.tile([1, V], F32R, tag="hb")
                    nc.sync.dma_start(hb[:],
                                      D["head_bf"][hd_][None, :].bitcast(F32R))
                    hwf = D["head_wf"][hd_].rearrange("(k p) v -> p k v",
                                                      p=128)
                    nv0 = 0
                    for nvi, nvsz in enumerate(VCH):
                        whc = phC.tile([128, HC, 414], F32R, tag="wh", bufs=2,
                                       name=f"wh{hd_}_{nvi}")
                        nc.sync.dma_start(whc[:, :, :nvsz],
                                          hwf[:, :, nv0:nv0 + nvsz])
                        for tci, (t0, tsz) in enumerate([(0, 128), (128, 72)]):
                            ot = phC.tile([128, VCH[0] ], F32, tag="ot",
                                          bufs=4, name=f"ot{hd_}_{nvi}_{tci}")
                            ps = ops.tile([128, 414], F32, tag="hmm", bufs=4)
                            for k in range(HC):
                                nc.tensor.matmul(ps[:tsz, :nvsz],
                                                 x1[:, k, t0:t0 + tsz],
                                                 whc[:, k, :nvsz],
                                                 start=(k == 0), stop=False)
                            nc.tensor.matmul(ps[:tsz, :nvsz],
                                             ones_r[0:1, t0:t0 + tsz],
                                             hb[:, nv0:nv0 + nvsz],
                                             start=False, stop=True)
                            nc.scalar.activation(ot[:tsz, :nvsz],
                                                 ps[:tsz, :nvsz], AF.Tanh)
                            nc.sync.dma_start(
                                outs[hd_][t0:t0 + tsz, nv0:nv0 + nvsz],
                                ot[:tsz, :nvsz])
                        nv0 += nvsz


def _ln_stats(tc, nc, sps, bca, sqp, ones_r, x_resid, ntok, eps_t,
              psum_tag="st"):
    """Mean/rstd over feature (partition) dim via all-ones matmuls.

    Returns broadcast tiles mean, rs of shape [128, ntok]."""
    nhalves = [(i * 300, min(300, ntok - i * 300)) for i in range((ntok + 299) // 300)]
    psS = [sps.tile([128, nsz], F32, tag=f"{psum_tag}S{i}", name=f"{psum_tag}S{i}", bufs=1)
           for i, (n0, nsz) in enumerate(nhalves)]
    psQ = [sps.tile([128, nsz], F32, tag=f"{psum_tag}Q{i}", name=f"{psum_tag}Q{i}", bufs=1)
           for i, (n0, nsz) in enumerate(nhalves)]
    HCn = x_resid.shape[1]
    for c in range(HCn):
        sq = sqp.tile([128, ntok], F32R, tag="sq", bufs=2)
        nc.scalar.activation(sq[:], x_resid[:, c, :], AF.Square)
        for i, (n0, nsz) in enumerate(nhalves):
            nc.tensor.matmul(psS[i][:], ones_r[:, :128],
                             x_resid[:, c, n0:n0 + nsz],
                             start=(c == 0), stop=(c == HCn - 1))
            nc.tensor.matmul(psQ[i][:], ones_r[:, :128], sq[:, n0:n0 + nsz],
                             start=(c == 0), stop=(c == HCn - 1))
    mean = bca.tile([128, ntok], F32, tag="mean")
    rs = bca.tile([128, ntok], F32, tag="rs")
    for i, (n0, nsz) in enumerate(nhalves):
        nsl = slice(n0, n0 + nsz)
        nc.scalar.mul(mean[:, nsl], psS[i][:], 1.0 / H)
        m2 = bca.tile([128, 300], F32, tag="m2", bufs=2)
        nc.scalar.activation(m2[:, :nsz], psS[i][:], AF.Square, scale=1.0 / H)
        vv = bca.tile([128, 300], F32, tag="vv", bufs=2)
        nc.vector.scalar_tensor_tensor(vv[:, :nsz], psQ[i][:], 1.0 / H,
                                       m2[:, :nsz], ALU.mult, ALU.subtract)
        sd = bca.tile([128, 300], F32, tag="sd", bufs=2)
        nc.scalar.activation(sd[:, :nsz], vv[:, :nsz], AF.Sqrt, bias=eps_t[:])
        nc.vector.reciprocal(rs[:, nsl], sd[:, :nsz])
    return mean, rs


# ====================== host side ======================

def _prep(inputs):
    """Host prep: per-core in_maps."""
    g = {k: np.asarray(v) for k, v in inputs.items()}
    f32 = np.float32

    word_emb = g["word_emb"].astype(f32)
    w_ih = g["gru_w_ih"].astype(f32)
    b_ih = g["gru_b_ih"].astype(f32)
    # vocab-sized input-transform table (weights-only precompute)
    xg_table = (word_emb @ w_ih.T + b_ih).astype(bf16)  # [V, 3H]

    te_full = g["time_emb"][g["timesteps"]]  # [B, S, H]
    ret_b = g["ret_b"].astype(f32)

    ln1_g, ln1_b = g["ln1_g"].astype(f32), g["ln1_b"].astype(f32)
    ln2_g, ln2_b = g["ln2_g"].astype(f32), g["ln2_b"].astype(f32)
    lnf_g, lnf_b = g["lnf_g"].astype(f32), g["lnf_b"].astype(f32)

    attn_wf = (g["attn_w"] * ln1_g[:, :, None]).astype(f32)
    attn_bf = (g["attn_b"] + np.einsum("lh,lhg->lg", ln1_b, g["attn_w"])).astype(f32)
    fc_wf = (g["fc_w"] * ln2_g[:, :, None]).astype(f32)
    fc_bf = (g["fc_b"] + np.einsum("lh,lhg->lg", ln2_b, g["fc_w"])).astype(f32)
    head_wf = (g["head_w"] * lnf_g[None, :, None]).astype(f32)
    head_bf = (g["head_b"] + np.einsum("h,khv->kv", lnf_b, g["head_w"])).astype(f32)

    tril = np.tril(np.ones((T, T), f32))  # [q, k]
    shared = {
        "w_hhT": np.ascontiguousarray(g["gru_w_hh"].astype(f32).T),
        "b_hh": g["gru_b_hh"].astype(f32),
        "ret_w": g["ret_w"].astype(f32).reshape(H),
        "wpeT": np.ascontiguousarray(g["wpe"][:T].astype(f32).T),
        "lne_g": g["ln_e_g"].astype(f32),
        "lne_b": g["ln_e_b"].astype(f32),
        "attn_wf": attn_wf, "attn_bf": attn_bf,
        "attn_pw16": g["attn_pw"].astype(bf16),
        "attn_pb": g["attn_pb"].astype(f32),
        "fc_wf": fc_wf, "fc_bf": fc_bf,
        "mlp_pw16": g["mlp_pw"].astype(bf16),
        "mlp_pb": g["mlp_pb"].astype(f32),
        "head_wf": head_wf, "head_bf": head_bf,
        "ones_r": np.ones((128, NGP), f32),
        "zeros_r": np.zeros((128, HC, NGP), f32),
        "ones_b": np.ones((128, 128), bf16),
        "ident_b": np.eye(128, dtype=bf16),
    }

    in_maps = []
    for c in range(NC_):
        bs = slice(c * BC, (c + 1) * BC)
        st = g["states"][bs]          # [4, S, TS]
        ac = g["actions"][bs]
        xg_s = np.ascontiguousarray(
            xg_table[st].transpose(2, 3, 0, 1).reshape(TS, G3, NG))
        xg_a = np.ascontiguousarray(
            xg_table[ac].transpose(2, 3, 0, 1).reshape(TA, G3, NG))
        te = te_full[bs].astype(f32)  # [4, S, H]
        teT = np.ascontiguousarray(te.transpose(2, 0, 1).reshape(H, NG))
        terT = np.ascontiguousarray(teT + ret_b[:, None])
        rtg = np.ascontiguousarray(
            g["returns_to_go"][bs, :, 0].astype(f32).reshape(1, NG))
        am3 = np.repeat(g["attention_mask"][bs], 3, axis=1).astype(f32)  # [4,150]
        maskT = (tril.T[None, :, :] * am3[:, :, None]).astype(bf16)  # [4, k, q]
        m = dict(shared)
        m.update({"xg_s": xg_s, "xg_a": xg_a, "teT": teT, "terT": terT,
                  "rtg": rtg, "maskT": np.ascontiguousarray(maskT)})
        in_maps.append(m)
    return in_maps


def kernel(**inputs):
    if "nc" not in _CACHED:
        _CACHED["nc"] = _build()
    nc = _CACHED["nc"]
    in_maps = _prep(inputs)
    res = run_bass_kernel_spmd(
        nc, in_maps, core_ids=list(range(NC_)),
        trace=bool(int(os.environ.get("DT_TRACE", "0"))))
    _CACHED["last"] = res
    outs = []
    for k in range(4):
        parts = [res.results[c][f"o{k}"].reshape(BC, S, V) for c in range(NC_)]
        outs.append(np.concatenate(parts, axis=0))
    return tuple(outs)


def bench(inputs, iters=3):
    """Steady-state wall time of the jitted 8-core NEFF exec with
    device-resident inputs (ns). NTFF tracing is unavailable under this
    axon client, so this is the HW-time proxy."""
    import time
    import jax
    from jax.sharding import Mesh, PartitionSpec, NamedSharding
    from jax.experimental.shard_map import shard_map
    from concourse import bass2jax, mybir as _mb
    from concourse.bass2jax import (_bass_exec_p, install_neuronx_cc_hook,
                                    partition_id_tensor)

    if "nc" not in _CACHED:
        _CACHED["nc"] = _build()
    nc = _CACHED["nc"]
    in_maps = _prep(inputs)
    install_neuronx_cc_hook()

    in_names, out_names, out_avals, zero_shapes = [], [], [], []
    for alloc in nc.m.functions[0].allocations:
        if not isinstance(alloc, _mb.MemoryLocationSet):
            continue
        name = alloc.memorylocations[0].name
        pname = (nc.partition_id_tensor.name if nc.partition_id_tensor
                 else None)
        if alloc.kind == "ExternalInput":
            if name != pname:
                in_names.append(name)
        elif alloc.kind == "ExternalOutput":
            out_names.append(name)
            shape = tuple(alloc.tensor_shape)
            dtype = _mb.dt.np(alloc.dtype)
            out_avals.append(jax.core.ShapedArray(shape, dtype))
            zero_shapes.append((shape, dtype))
    n_params = len(in_names)
    n_outs = len(out_avals)
    all_names = in_names + out_names
    if nc.partition_id_tensor:
        all_names = all_names + [nc.partition_id_tensor.name]
    donate = tuple(range(n_params, n_params + n_outs))

    def _body(*args):
        operands = list(args)
        if nc.partition_id_tensor:
            operands.append(partition_id_tensor())
        return tuple(_bass_exec_p.bind(
            *operands, out_avals=tuple(out_avals), in_names=tuple(all_names),
            out_names=tuple(out_names), lowering_input_output_aliases=(),
            sim_require_finite=True, sim_require_nnan=True, nc=nc))

    devices = jax.devices()[:NC_]
    mesh = Mesh(np.asarray(devices), ("core",))
    spec = PartitionSpec("core")
    sharded = jax.jit(
        shard_map(_body, mesh=mesh, in_specs=(spec,) * (n_params + n_outs),
                  out_specs=(spec,) * n_outs, check_rep=False),
        donate_argnums=donate, keep_unused=True)

    sh = NamedSharding(mesh, spec)
    dev_in = [jax.device_put(
        np.concatenate([np.asarray(in_maps[c][n]) for c in range(NC_)], axis=0),
        sh) for n in in_names]
    zeros_sets = [
        [jax.device_put(np.zeros((NC_ * s0[0], *s0[1:]), dt0), sh)
         for s0, dt0 in zero_shapes]
        for _ in range(iters + 1)
    ]
    # warmup (compiles)
    outs = sharded(*dev_in, *zeros_sets[0])
    jax.block_until_ready(outs)
    best = None
    for i in range(iters):
        t0 = time.perf_counter()
        outs = sharded(*dev_in, *zeros_sets[i + 1])
        jax.block_until_ready(outs)
        dt = time.perf_counter() - t0
        best = dt if best is None else min(best, dt)
    _CACHED["bench_outs"] = [np.asarray(o) for o in outs]
    return int(best * 1e9)

